# revision 3
# baseline (speedup 1.0000x reference)
"""MoE router gate v4 — merged hi|lo layout for 8KB DMA descriptors.

logits = x @ W.T ([32768,2048] @ [2048,64]); outputs top-6 indices (int32)
and pre-softmax logits (fp32) at those indices, per token, in top_k order.

Over v3:
  - x ships as ONE tensor xm[KC, 128, 2*T]: per (chunk, partition) row is
    [hi fp16 for the group's 2048 tokens | lo fp16 for the same tokens],
    8KB contiguous -> one DMA per (group, chunk) at the measured ~460GB/s
    8KB-descriptor rate (vs ~400 at 4KB, ~250 at 2KB).
  - Chain order is block-local: for each 4-chunk block, 16 hi then 16 lo
    matmuls, so a block's SBUF tile lives only ~7us (bufs=4 prefetch).
  - Stationary per chunk is [wh_c | wl_c] (col-tiled 128 wide): PSUM rows
    0:64 accumulate x*wh, rows 64:128 x*wl — all four fp16-split terms in
    2 logical passes. Transpose matmuls fold the halves during top-k.

Per-core roofline: DMA ~73us (bound); PE ~52us; DVE ~17us.
"""
import sys

sys.path.insert(0, "/opt/trn_rl_repo")

import numpy as np

T_FULL, D, E = 32768, 2048, 64
N_CORES = 8
T_SHARD = T_FULL // N_CORES


def build_gate(T=T_SHARD, TQ=2048, CB=1, XB=0, reps=1, nomm=0, notopk=0, roll=0):
    import concourse.bacc as bacc
    import concourse.bass as bass
    import concourse.mybir as mybir
    import concourse.tile as tile
    from concourse import masks

    f32 = mybir.dt.float32
    f16 = mybir.dt.float16
    u32 = mybir.dt.uint32

    KC = D // 128        # 16 contraction chunks
    NG = T // TQ         # token groups
    NJ = TQ // 512       # 512-token psum banks per group
    NS = TQ // 128       # 128-token subtiles per group
    ST = T // 128        # subtiles per shard
    NCB = KC // CB       # chunk blocks per group
    W2 = 2 * TQ          # merged hi|lo row length per group

    nc = bacc.Bacc("TRN2", target_bir_lowering=False)
    xm_d = nc.dram_tensor("xm", [KC, 128, 2 * T], f16, kind="ExternalInput")
    w2_d = nc.dram_tensor("w2", [128, KC * 128], f16, kind="ExternalInput")
    ow_d = nc.dram_tensor("ow", [128, ST * 8], f32, kind="ExternalOutput")
    oi_d = nc.dram_tensor("oi", [128, ST * 8], u32, kind="ExternalOutput")

    with tile.TileContext(nc) as tc:
        with (
            tc.tile_pool(name="const", bufs=1) as constp,
            tc.tile_pool(name="xm", bufs=XB or (5 if CB == 4 else 18)) as xmp,
            tc.tile_pool(name="lg", bufs=2) as lgp,
            tc.tile_pool(name="ltsb", bufs=3) as ltsbp,
            tc.tile_pool(name="outs", bufs=1) as outp,
            tc.tile_pool(name="ps", bufs=5, space=bass.MemorySpace.PSUM) as psp,
            tc.tile_pool(name="lt", bufs=3, space=bass.MemorySpace.PSUM) as ltp,
        ):
            ident = constp.tile([128, 128], f32)
            masks.make_identity(nc, ident[:])
            w2_sb = constp.tile([128, KC * 128], f16)
            nc.sync.dma_start(out=w2_sb[:], in_=w2_d[:])
            ow_sb = outp.tile([128, ST * 8], f32)
            oi_sb = outp.tile([128, ST * 8], u32)
            if nomm or notopk:
                nc.vector.memset(ow_sb[:], 0.0)
                nc.vector.memset(oi_sb[:], 0)

            def topk_bank(lgh, lgl, g, j):
                # lt = T(hi block) + T(lo block): the transpose matmuls also
                # fold the split halves, avoiding a PSUM+PSUM elementwise op
                for s in range(j * 4, (j + 1) * 4):
                    st = g * NS + s
                    lt = ltp.tile([128, E], f32, tag="lt")
                    nc.tensor.matmul(
                        lt[:],
                        lgh[:, s * 128:(s + 1) * 128],
                        ident[0:64, 0:64],
                        is_transpose=True,
                        start=True,
                        stop=False,
                    )
                    nc.tensor.matmul(
                        lt[:],
                        lgl[:, s * 128:(s + 1) * 128],
                        ident[0:64, 0:64],
                        is_transpose=True,
                        start=False,
                        stop=True,
                    )
                    ltsb = ltsbp.tile([128, E], f32, tag="ltsb")
                    nc.vector.tensor_copy(ltsb[:], lt[:])
                    nc.vector.max(ow_sb[:, st * 8:(st + 1) * 8], ltsb[:])
                    nc.vector.max_index(
                        oi_sb[:, st * 8:(st + 1) * 8],
                        ow_sb[:, st * 8:(st + 1) * 8],
                        ltsb[:],
                    )

            def body():
                for g in range(NG):
                    eng = nc.sync
                    blocks = []
                    for b in range(NCB):
                        xb = xmp.tile([128, CB, W2], f16, tag="xm", name="xb")
                        blocks.append(xb)
                        for i in range(CB):
                            c = b * CB + i
                            eng.dma_start(
                                out=xb[:, i, :],
                                in_=xm_d[c, :, g * W2:(g + 1) * W2],
                            )

                    pss = []
                    for j in range(NJ):
                        ps_j = psp.tile([128, 512], f32, tag="ps", name="ps_j")
                        pss.append(ps_j)
                    # block-local hi then lo: one 32*NJ-matmul chain per bank,
                    # PSUM rows 0:64 = x*wh, 64:128 = x*wl
                    if not nomm:
                        phs = (1, 0) if roll & 1 else (0, 1)
                        irng = (
                            tuple(reversed(range(CB))) if roll & 2 else tuple(range(CB))
                        )

                        def mm(b, ph, i, j):
                            c = b * CB + i
                            o0 = ph * TQ
                            nc.tensor.matmul(
                                pss[j][:],
                                w2_sb[:, c * 128:(c + 1) * 128],
                                blocks[b][:, i, o0 + j * 512:o0 + (j + 1) * 512],
                                start=(b == 0 and ph == phs[0] and i == irng[0]),
                                stop=(b == NCB - 1 and ph == phs[1] and i == irng[-1]),
                            )

                        lgh = lgp.tile([64, TQ], f32, tag="lgh", name="lgh")
                        lgl = lgp.tile([64, TQ], f32, tag="lgl", name="lgl")

                        for b in range(NCB - 1):
                            for ph in phs:
                                for i in irng:
                                    for j in range(NJ):
                                        mm(b, ph, i, j)
                        # last block bank-major: bank j's chain closes after
                        # its 2*CB matmuls, so evac+topk overlap later banks
                        b = NCB - 1
                        for j in range(NJ):
                            for ph in phs:
                                for i in irng:
                                    mm(b, ph, i, j)
                            o = j * 512
                            nc.vector.tensor_copy(lgh[:, o:o + 512], pss[j][0:64, :])
                            nc.scalar.copy(lgl[:, o:o + 512], pss[j][64:128, :])
                            if not notopk:
                                topk_bank(lgh, lgl, g, j)

                nc.sync.dma_start(out=ow_d[:], in_=ow_sb[:])
                nc.sync.dma_start(out=oi_d[:], in_=oi_sb[:])

            if reps == 1:
                body()
            else:
                with tc.For_i(0, reps):
                    body()

    nc.compile()
    return nc


# Exact power-of-2 pre-scaling before the fp16 split: without it the lo
# halves (|W| ~ 0.02 -> wl ~ 5e-6) fall below fp16's 6.1e-5 min-normal and
# get subnormal-quantized, a fixed ~1e-6-scale logit bias that flips
# near-tie top-k rows. Scaling is exact (power of 2), outputs unscale by
# 2^-(KX+KW) exactly on the host.
KX, KW = 6, 8

def shard_inputs(x, W, TQ=2048):
    x = np.asarray(x, dtype=np.float32) * np.float32(2.0 ** KX)
    W = np.asarray(W, dtype=np.float32) * np.float32(2.0 ** KW)
    xt = np.ascontiguousarray(x.T)            # [D, T]
    xh = xt.astype(np.float16)
    xl = (xt - xh.astype(np.float32)).astype(np.float16)
    wt = np.ascontiguousarray(W.T)            # [D, E]
    wh = wt.astype(np.float16)
    wl = (wt - wh.astype(np.float32)).astype(np.float16)
    KC = D // 128
    w2 = np.concatenate(
        [wh.reshape(KC, 128, E), wl.reshape(KC, 128, E)], axis=2
    )  # [KC, 128, 128]
    w2 = np.ascontiguousarray(w2.transpose(1, 0, 2).reshape(128, KC * 128))
    NG = T_SHARD // TQ
    maps = []
    for i in range(N_CORES):
        sl = slice(i * T_SHARD, (i + 1) * T_SHARD)
        hi = xh[:, sl].reshape(KC, 128, NG, TQ)
        lo = xl[:, sl].reshape(KC, 128, NG, TQ)
        xm = np.concatenate([hi, lo], axis=3).reshape(KC, 128, 2 * T_SHARD)
        maps.append({"xm": np.ascontiguousarray(xm), "w2": w2})
    return maps


def unshard_outputs(results):
    ST = T_SHARD // 128
    idxs, wts = [], []
    for r in results:
        ow = r["ow"].reshape(128, ST, 8).transpose(1, 0, 2)[:, :, :6]
        oi = r["oi"].reshape(128, ST, 8).transpose(1, 0, 2)[:, :, :6]
        wts.append(np.ascontiguousarray(ow.reshape(T_SHARD, 6)) * np.float32(2.0 ** -(KX + KW)))
        idxs.append(oi.astype(np.int32).reshape(T_SHARD, 6))
    return np.concatenate(idxs, 0), np.concatenate(wts, 0)


_CACHE = {}


def _get_nc():
    if "nc" not in _CACHE:
        from concourse.bass_interp import get_hw_module

        nc = build_gate()
        nc.m = get_hw_module(nc.m)
        _CACHE["nc"] = nc
    return _CACHE["nc"]


def run_sharded(x, W, trace=False):
    from concourse.bass_utils import run_bass_kernel_spmd

    nc = _get_nc()
    res = run_bass_kernel_spmd(
        nc, shard_inputs(x, W), core_ids=list(range(N_CORES)), trace=trace
    )
    idx, wts = unshard_outputs(res.results)
    return res, idx, wts


def kernel(x, W):
    _, idx, wts = run_sharded(x, W, trace=False)
    return idx, wts
